# revision 4
# baseline (speedup 1.0000x reference)
"""Trainium2 Bass kernel for GQA attention (B=2, S=2048, D=2048, 16 q-heads /
4 kv-heads, HD=128) with per-head QK RMSNorm + RoPE + causal softmax + output
projection.

Sharding: 8 cores = (batch b in {0,1}) x (kv-group g in {0..3}). Each core
computes its batch's 4 q-heads + 1 kv-head and a partial output through the
row-sharded Wo; the host sums the 4 partials per batch.

Phase 2 computes scores TRANSPOSED (S^T = K^T q, k on partitions) so the
softmax probabilities feed the P@V matmul directly as the moving operand --
no PE transposes. The softmax denominator comes from a ones-matmul
(broadcast column-sum of exp(S^T)), reciprocal on DVE, and the normalization
is applied to the PV result. The output projection is interleaved into the
attention instruction stream to keep the PE busy while the scalar engine
computes exponentials.
"""
import numpy as np

import concourse.bass as bass  # noqa: F401
import concourse.mybir as mybir
import concourse.tile as tile
from concourse import bacc
from concourse.bass_utils import run_bass_kernel_spmd

F32 = mybir.dt.float32
F16 = mybir.dt.float16
AF = mybir.ActivationFunctionType
OP = mybir.AluOpType

B, S, D = 2, 2048, 2048
NH, NKV, HD = 16, 4, 128
REP = NH // NKV
EPS = 1e-6
EXPB = -5.0  # exp bias: cancels in softmax, keeps exp() in fp16 range


def build(s=S):
    """Build + compile the per-core SPMD program (identical on all 8 cores)."""
    sc = s // 128          # s-chunks
    kc = D // 128          # contraction chunks
    nqb = sc // 4          # q superblocks (512 wide)
    nc = bacc.Bacc("TRN2", target_bir_lowering=False, debug=False, num_devices=8)

    # host pre-tiled so every DMA is 128 partition-contiguous descriptors
    xt_d = nc.dram_tensor("xtl", [sc, 128, kc, 128], F16, kind="ExternalInput")
    wqkv_d = nc.dram_tensor("wqkv", [128, kc, 768], F16, kind="ExternalInput")
    wo_d = nc.dram_tensor("wo", [128, REP, D], F16, kind="ExternalInput")
    ropes_d = nc.dram_tensor("ropes", [s, 4 * HD], F32, kind="ExternalInput")
    trit_d = nc.dram_tensor("trit", [128, 128], F16, kind="ExternalInput")
    iden16_d = nc.dram_tensor("ident16", [128, 128], F16, kind="ExternalInput")
    out_d = nc.dram_tensor("outp", [s, D], F16, kind="ExternalOutput")

    with tile.TileContext(nc) as tc:
        with tc.tile_pool(name="pers", bufs=1) as pers:
            qT = pers.tile([128, REP, s], F16, tag="qT")
            kT = pers.tile([128, s], F16, tag="kT")
            vv = pers.tile([128, sc, HD], F16, tag="vv")
            aoT = pers.tile([128, REP, s], F16, tag="aoT")
            trit_t = pers.tile([128, 128], F16, tag="trit")
            iden16_t = pers.tile([128, 128], F16, tag="ident16")
            nc.gpsimd.dma_start(out=trit_t[:], in_=trit_d[:, :])
            nc.gpsimd.dma_start(out=iden16_t[:], in_=iden16_d[:, :])
            eps_t = pers.tile([128, 1], F32, tag="eps")
            nc.vector.memset(eps_t[:], EPS)
            expb_t = pers.tile([128, 1], F32, tag="expb")
            nc.vector.memset(expb_t[:], EXPB)
            ones_t = pers.tile([128, 128], F16, tag="ones")
            nc.vector.memset(ones_t[:], 1.0)

            # ---------------- Phase 1: QKV + RMSNorm + RoPE -----------------
            with (
                tc.tile_pool(name="wq", bufs=1) as wq,
                tc.tile_pool(name="xp", bufs=3) as xp,
                tc.tile_pool(name="cp", bufs=3) as cp,
                tc.tile_pool(name="st", bufs=3) as st,
                tc.tile_pool(name="psA", bufs=2, space="PSUM") as psA,
                tc.tile_pool(name="psB", bufs=2, space="PSUM") as psB,
                tc.tile_pool(name="psT", bufs=2, space="PSUM") as psT,
            ):
                wqkv_t = wq.tile([128, kc, 768], F16, tag="wqkv")
                ropes_r = ropes_d.rearrange("(m si) h -> si m h", si=128)

                for k in range(kc):
                    nc.sync.dma_start(out=wqkv_t[:, k], in_=wqkv_d[:, k])

                for m in range(sc):
                    xt = xp.tile([128, kc, 128], F16, tag="xt")
                    nc.gpsimd.dma_start(out=xt[:], in_=xt_d[m])
                    cst = cp.tile([128, 512], F32, tag="cst")
                    nc.gpsimd.dma_start(out=cst[:], in_=ropes_r[:, m])
                    cq = cst[:, 0:128]
                    sq_ = cst[:, 128:256]
                    ck = cst[:, 256:384]
                    sk_ = cst[:, 384:512]

                    pqt = psA.tile([128, 1024], F32, tag="psA")
                    pq = pqt[:, 0:512]
                    pkv = psB.tile([128, 512], F32, tag="psB")
                    for k in range(kc):
                        nc.tensor.matmul(
                            pq, xt[:, k], wqkv_t[:, k, 0:512],
                            start=(k == 0), stop=(k == kc - 1),
                        )
                    for k in range(kc):
                        nc.tensor.matmul(
                            pkv[:, 0:256], xt[:, k], wqkv_t[:, k, 512:768],
                            start=(k == 0), stop=(k == kc - 1),
                        )

                    # ---- batched RMSNorm stats: one Square per q block ----
                    ss = st.tile([128, 16], F32, tag="ss")
                    sqs = st.tile([128, 512], F32, tag="sqs")
                    nc.scalar.activation(sqs[:], pq, AF.Square)
                    sqk = st.tile([128, 128], F32, tag="sqk")
                    nc.scalar.activation(
                        sqk[:], pkv[:, 0:128], AF.Square, accum_out=ss[:, 4:5],
                    )
                    nc.vector.tensor_reduce(
                        out=ss[:, 0:4],
                        in_=sqs[:].rearrange("p (h d) -> p h d", d=128),
                        axis=mybir.AxisListType.X, op=OP.add,
                    )
                    nc.scalar.activation(
                        ss[:, 8:13], ss[:, 0:5], AF.Sqrt,
                        bias=eps_t[:], scale=1.0 / HD,
                    )
                    rs = st.tile([128, 8], F32, tag="rs")
                    nc.vector.reciprocal(rs[:, 0:5], ss[:, 8:13])

                    # ---- fused RoPE for all 4 q heads (broadcast APs) ----
                    pq3 = pq.rearrange("p (h d) -> p h d", d=128)
                    u = st.tile([128, REP, 128], F32, tag="u")
                    nc.vector.tensor_mul(
                        u[:], pq3,
                        rs[:, 0:4].rearrange("p (h o) -> p h o", o=1).broadcast_to(
                            [128, REP, 128]),
                    )
                    qn = st.tile([128, 512], F16, tag="qn")
                    qn3 = qn[:].rearrange("p (h d) -> p h d", d=128)
                    ra = st.tile([128, REP, 128], F32, tag="ra")
                    nc.vector.tensor_mul(
                        ra[:], u[:],
                        cq.rearrange("p (o d) -> p o d", o=1).broadcast_to(
                            [128, REP, 128]),
                    )
                    rb = st.tile([128, REP, 128], F32, tag="rb")
                    nc.vector.tensor_mul(
                        rb[:, :, 0:64], u[:, :, 64:128],
                        sq_[:, 0:64].rearrange("p (o d) -> p o d", o=1).broadcast_to(
                            [128, REP, 64]),
                    )
                    nc.vector.tensor_mul(
                        rb[:, :, 64:128], u[:, :, 0:64],
                        sq_[:, 64:128].rearrange("p (o d) -> p o d", o=1).broadcast_to(
                            [128, REP, 64]),
                    )
                    nc.vector.tensor_add(qn3, ra[:], rb[:])

                    # ---- k head rope ----
                    uk = st.tile([128, 128], F32, tag="uk")
                    nc.vector.tensor_scalar_mul(uk[:], pkv[:, 0:128], rs[:, 4:5])
                    kn = st.tile([128, 128], F16, tag="kn")
                    rak = st.tile([128, 128], F32, tag="rak")
                    nc.vector.tensor_mul(rak[:], uk[:], ck)
                    rbk = st.tile([128, 128], F32, tag="rbk")
                    nc.vector.tensor_mul(rbk[:, 0:64], uk[:, 64:128], sk_[:, 0:64])
                    nc.vector.tensor_mul(rbk[:, 64:128], uk[:, 0:64], sk_[:, 64:128])
                    nc.vector.tensor_add(kn[:], rak[:], rbk[:])

                    # ---- transposes to head-major ----
                    for h in range(REP):
                        pt = psT.tile([128, 512], F16, tag="psT")
                        nc.tensor.transpose(
                            pt[:, 0:128], qn[:, h * 128:(h + 1) * 128], iden16_t[:],
                        )
                        nc.vector.tensor_copy(
                            out=qT[:, h, m * 128:(m + 1) * 128], in_=pt[:, 0:128],
                        )
                    pt = psT.tile([128, 512], F16, tag="psT")
                    nc.tensor.transpose(pt[:, 0:128], kn[:], iden16_t[:])
                    nc.vector.tensor_copy(
                        out=kT[:, m * 128:(m + 1) * 128], in_=pt[:, 0:128],
                    )
                    nc.vector.tensor_copy(out=vv[:, m, :], in_=pkv[:, 128:256])

            # ------- Phase 2: transposed-score attention + fused out-proj ---
            with (
                tc.tile_pool(name="wop", bufs=1) as wop,
                tc.tile_pool(name="ep", bufs=3) as ep,
                tc.tile_pool(name="ac", bufs=2) as ac,
                tc.tile_pool(name="bx", bufs=2) as bx,
                tc.tile_pool(name="ob", bufs=2) as ob,
                tc.tile_pool(name="stp", bufs=2, space="PSUM") as stp,
                tc.tile_pool(name="pvp", bufs=2, space="PSUM") as pvp,
                tc.tile_pool(name="pop", bufs=2, space="PSUM") as pop,
            ):
                wo_t = wop.tile([128, REP, D], F16, tag="wo")
                nc.sync.dma_start(out=wo_t[:], in_=wo_d[:, :, :])

                # out-projection units (m, n) pending; drained between
                # attention groups so the PE never starves while the scalar
                # engine runs exp.
                pending = []
                ot_tiles = {}

                def drain_one():
                    if not pending:
                        return
                    m, n = pending.pop(0)
                    if n == 0:
                        ot_tiles[m] = ob.tile([128, D], F16, tag="ot",
                                              name="ot")
                    ot = ot_tiles[m]
                    po = pop.tile([128, 512], F32, tag="po")
                    for e in range(REP):
                        nc.tensor.matmul(
                            po[:], aoT[:, e, m * 128:(m + 1) * 128],
                            wo_t[:, e, n * 512:(n + 1) * 512],
                            start=(e == 0), stop=(e == REP - 1),
                        )
                    if n % 2 == 0:
                        nc.vector.tensor_copy(
                            out=ot[:, n * 512:(n + 1) * 512], in_=po[:],
                        )
                    else:
                        nc.scalar.copy(
                            out=ot[:, n * 512:(n + 1) * 512], in_=po[:],
                        )
                    if n == REP - 1:
                        nc.sync.dma_start(
                            out=out_d[m * 128:(m + 1) * 128, :], in_=ot[:],
                        )
                        del ot_tiles[m]

                for Qb in range(nqb):
                    q0 = Qb * 512
                    J = 4 * Qb + 4
                    # groups of k-chunks sharing one ST psum tile + one exp:
                    # full-width pairs below the diagonal, then the 4 ragged
                    # diagonal chunks paired (512+384, 256+128).
                    groups = [(jj, jj + 1) for jj in range(0, 4 * Qb, 2)]
                    groups += [(4 * Qb, 4 * Qb + 1), (4 * Qb + 2, 4 * Qb + 3)]
                    for h in range(REP):
                        pv = pvp.tile([128, 512], F32, tag="pv")
                        acc = ac.tile([128, 512], F16, tag="acc")
                        first = True
                        for g in groups:
                            spans = []
                            tw = 0
                            for j in g:
                                lo = max(0, j * 128 - q0)
                                w = 512 - lo
                                spans.append((j, tw, lo, w))
                                tw += w
                            stt = stp.tile([128, 1024], F32, tag="st")
                            for j, off, lo, w in spans:
                                nc.tensor.matmul(
                                    stt[:, off:off + w],
                                    kT[:, j * 128:(j + 1) * 128],
                                    qT[:, h, q0 + lo:q0 + 512],
                                    start=True, stop=True,
                                )
                            et = ep.tile([128, 1024], F16, tag="et")
                            nc.scalar.activation(
                                et[:, 0:tw], stt[:, 0:tw], AF.Exp, bias=expb_t[:],
                            )
                            for j, off, lo, w in spans:
                                if j * 128 >= q0:  # diagonal chunk: 0/1 mask
                                    nc.vector.tensor_mul(
                                        et[:, off:off + 128],
                                        et[:, off:off + 128], trit_t[:],
                                    )
                            for j, off, lo, w in spans:
                                if first:
                                    nc.vector.tensor_copy(
                                        out=acc[:, lo:512], in_=et[:, off:off + w],
                                    )
                                    first = False
                                else:
                                    nc.vector.tensor_add(
                                        acc[:, lo:512], acc[:, lo:512],
                                        et[:, off:off + w],
                                    )
                            for j, off, lo, w in spans:
                                nc.tensor.matmul(
                                    pv[:, lo:512], vv[:, j, :], et[:, off:off + w],
                                    start=(j == 0), stop=(j == J - 1),
                                    skip_group_check=True,
                                )
                            drain_one()
                        # denominator: broadcast column-sum via ones-matmul,
                        # then reciprocal + normalize the PV result.
                        bcs = pop.tile([128, 512], F32, tag="po")
                        nc.tensor.matmul(bcs[:], ones_t[:], acc[:],
                                         start=True, stop=True)
                        bcsr = bx.tile([128, 512], F32, tag="bcsr")
                        nc.vector.reciprocal_approx_fast(out=bcsr[:], in_=bcs[:])
                        nc.vector.tensor_mul(
                            aoT[:, h, q0:q0 + 512], pv[:], bcsr[:],
                        )
                        drain_one()
                    for m in range(4 * Qb, 4 * Qb + 4):
                        for n in range(REP):
                            pending.append((m, n))
                while pending:
                    drain_one()

    nc.compile()
    return nc


def make_in_maps(x, cos, sin, Wq, Wk, Wv, Wo, q_norm_w, k_norm_w):
    qsc = (q_norm_w / np.sqrt(HD)).astype(np.float32)
    ksc = k_norm_w.astype(np.float32)

    def rope_consts(w):
        cw = (cos * w[None, :]).astype(np.float32)
        sw = np.empty_like(cw)
        sw[:, :64] = -sin[:, :64] * w[None, 64:]
        sw[:, 64:] = sin[:, 64:] * w[None, :64]
        return cw, sw

    cwq, swq = rope_consts(qsc)
    cwk, swk = rope_consts(ksc)
    ropes = np.ascontiguousarray(np.concatenate([cwq, swq, cwk, swk], axis=1))
    r = np.arange(128)
    trit = (r[:, None] <= r[None, :]).astype(np.float16)  # k <= q
    ident16 = np.eye(128, dtype=np.float16)

    in_maps = []
    for c in range(8):
        b, g = c // 4, c % 4
        # x^T tiled [m, ki, dk, t] so DMAs are partition-contiguous
        xtl = np.ascontiguousarray(
            x[b].T.astype(np.float16).reshape(D // 128, 128, S // 128, 128)
            .transpose(2, 1, 0, 3)
        )
        wqkv = np.concatenate(
            [
                Wq[:, g * 512:(g + 1) * 512],
                Wk[:, g * 128:(g + 1) * 128],
                Wv[:, g * 128:(g + 1) * 128],
            ],
            axis=1,
        ).astype(np.float16)
        wqkv = np.ascontiguousarray(
            wqkv.reshape(D // 128, 128, 768).transpose(1, 0, 2)
        )
        wo = Wo[g * 512:(g + 1) * 512, :].astype(np.float16)
        wo = np.ascontiguousarray(wo.reshape(REP, 128, D).transpose(1, 0, 2))
        in_maps.append(
            dict(
                xtl=xtl, wqkv=wqkv, wo=wo, ropes=ropes,
                trit=trit, ident16=ident16,
            )
        )
    return in_maps


_cached = None


def kernel(x, cos, sin, Wq, Wk, Wv, Wo, q_norm_w, k_norm_w):
    global _cached
    x = np.asarray(x, np.float32)
    cos = np.asarray(cos, np.float32)
    sin = np.asarray(sin, np.float32)
    in_maps = make_in_maps(
        x, cos, sin,
        np.asarray(Wq, np.float32), np.asarray(Wk, np.float32),
        np.asarray(Wv, np.float32), np.asarray(Wo, np.float32),
        np.asarray(q_norm_w, np.float32), np.asarray(k_norm_w, np.float32),
    )
    if _cached is None:
        _cached = build()
    res = run_bass_kernel_spmd(_cached, in_maps, core_ids=list(range(8)))
    out = np.zeros((B, S, D), np.float64)
    for c in range(8):
        out[c // 4] += res.results[c]["outp"].astype(np.float64)
    return out.astype(np.float32)
